# revision 1
# baseline (speedup 1.0000x reference)
"""Trainium2 Bass kernel for the CapibaraByte recurrent-scan problem.

Reference computation (B=128, T=1024, D_IN=256, H=2048):
    conv = einsum('btd,dh->bth', x, W_conv)
    step:  s <- 0.9*s + 0.1*gelu(s @ W_state + conv[:,t] + bias)
    out = (s @ W_state + bias, s)

Strategy: data-parallel over batch across 8 cores (B_local=16/core); the
scan runs fully on-core with zero cross-core traffic.  The per-step GEMM
(16 x 2048) @ (2048 x 2048) is done state-stationary (state as the PE
weights) with 4-way column tiling so four 16-wide weight tiles stream
concurrently on disjoint PE column groups.  The state lives in transposed
[h, b] layout; each step the packed [b, h] matmul output is transposed
back with 16 PE transpose ops.  All matmul operands are bf16 (fp32 PSUM
accumulate); a fp32 master copy of the state keeps the blend exact.  The
x @ W_conv projection is fused into the loop in T-blocks so conv features
never round-trip DRAM.
"""

import sys

for _p in ("/opt/trn_rl_repo",):
    if _p not in sys.path:
        sys.path.insert(0, _p)

import numpy as np
import ml_dtypes

import concourse.bass as bass
import concourse.tile as tile
from concourse import bacc, mybir
from concourse.bass import ds
from concourse.bass_utils import run_bass_kernel_spmd

AFT = mybir.ActivationFunctionType
ALU = mybir.AluOpType
F32 = mybir.dt.float32
BF16 = mybir.dt.bfloat16

B, T_FULL, D_IN, H = 128, 1024, 256, 2048
NCORES = 8
BL = B // NCORES            # 16 batch rows per core
KT = H // 128               # 16 contraction tiles
MT = H // 128               # 16 output h-tiles
UPDATE = 0.1


def build(T_steps=T_FULL, U=8, act=AFT.Gelu_apprx_tanh):
    """Build the Bacc graph for a T_steps-long scan, U steps per loop iter."""
    assert T_steps % U == 0
    nc = bacc.Bacc("TRN2", target_bir_lowering=False, debug=False,
                   num_devices=NCORES)

    xT_d = nc.dram_tensor("xT", [2, 128, T_steps * BL], BF16,
                          kind="ExternalInput").ap()
    w_d = nc.dram_tensor("w_arr", [128, KT * H], BF16,
                         kind="ExternalInput").ap()
    wc_d = nc.dram_tensor("wc_arr", [128, 2 * H], BF16,
                          kind="ExternalInput").ap()
    biasA_d = nc.dram_tensor("bias_arr", [128, MT], F32,
                             kind="ExternalInput").ap()
    biasT_d = nc.dram_tensor("bias_bcT", [128, MT * BL], F32,
                             kind="ExternalInput").ap()
    ident_d = nc.dram_tensor("ident", [128, 128], F32,
                             kind="ExternalInput").ap()
    outT_d = nc.dram_tensor("outT", [128, MT * BL], F32,
                            kind="ExternalOutput").ap()
    stT_d = nc.dram_tensor("stT", [128, MT * BL], F32,
                           kind="ExternalOutput").ap()

    UB = U * BL  # conv block column count per k-tile

    with tile.TileContext(nc) as tc:
        with (
            tc.tile_pool(name="persist", bufs=1) as persist,
            tc.tile_pool(name="xin", bufs=2) as xpool,
            tc.tile_pool(name="cblk", bufs=2) as cpool,
            tc.tile_pool(name="work", bufs=2) as work,
            tc.tile_pool(name="psum_su", bufs=1, space="PSUM") as psum_su,
            tc.tile_pool(name="psum_t", bufs=2, space="PSUM") as psum_t,
            tc.tile_pool(name="psum_c", bufs=2, space="PSUM") as psum_c,
        ):
            # ---- resident tensors ----
            w_sb = persist.tile([128, KT * H], BF16, tag="w_sb")
            nc.sync.dma_start(w_sb[:], w_d[:])
            wc_sb = persist.tile([128, 2 * H], BF16, tag="wc_sb")
            nc.sync.dma_start(wc_sb[:], wc_d[:])
            bias_sb = persist.tile([128, MT], F32, tag="bias_sb")
            nc.sync.dma_start(bias_sb[:], biasA_d[:])
            biasT_sb = persist.tile([128, MT * BL], F32, tag="biasT_sb")
            nc.sync.dma_start(biasT_sb[:], biasT_d[:])
            ident_sb = persist.tile([128, 128], F32, tag="ident_sb")
            nc.sync.dma_start(ident_sb[:], ident_d[:])

            # state in [h, b] layout: col tau*BL+b, partition p -> h=128*tau+p
            stT_bf = persist.tile([128, MT * BL], BF16, tag="stT_bf")
            nc.vector.memset(stT_bf[:], 0.0)
            stT_f32 = persist.tile([128, MT * BL], F32, tag="stT_f32")
            nc.vector.memset(stT_f32[:], 0.0)

            def mm_phase():
                """64 col-tiled matmuls: packed su[32g+b, n] for n-chunk g."""
                sus = [psum_su.tile([128, 512], F32, tag=f"su{g}", name=f"su{g}")
                       for g in range(4)]
                for k in range(KT):
                    lhs = stT_bf[:, BL * k:BL * (k + 1)]
                    for g in range(4):
                        nc.tensor.matmul(
                            sus[g][32 * g:32 * g + BL, :],
                            lhsT=lhs,
                            rhs=w_sb[:, k * H + 512 * g:k * H + 512 * (g + 1)],
                            start=(k == 0), stop=(k == KT - 1),
                            tile_position=(0, 32 * g),
                        )
                return sus

            def evict_transpose(sus):
                """psum (packed [b,h]) -> base-0 sbuf f32 -> PE-transpose to [h,b].

                Transpose inputs must sit at partition base 0 (walrus rejects
                offset-partition transpose), so the evictions shift partitions
                32g -> 0 while copying.
                """
                su16 = work.tile([BL, H], F32, tag="su16")
                for g in range(4):
                    src = sus[g][32 * g:32 * g + BL, :]
                    dst = su16[:, 512 * g:512 * (g + 1)]
                    if g % 2 == 0:
                        nc.vector.tensor_copy(dst, src)
                    else:
                        nc.scalar.copy(dst, src)
                pT = psum_t.tile([128, MT * BL], F32, tag="pT")
                for tau in range(MT):
                    nc.tensor.matmul(
                        pT[:, BL * tau:BL * (tau + 1)],
                        lhsT=su16[:, 128 * tau:128 * (tau + 1)],
                        rhs=ident_sb[0:BL, 0:BL],
                        is_transpose=True, start=True, stop=True,
                    )
                return pT

            def do_step(cT, u):
                sus = mm_phase()
                pT = evict_transpose(sus)
                # su_T + (c_t + bias)   [c was biased at conv eviction]
                s1 = work.tile([128, MT * BL], F32, tag="s1")
                nc.vector.tensor_tensor(
                    s1[:], pT[:], cT[:, u * (MT * BL):(u + 1) * (MT * BL)],
                    ALU.add)
                gsb = work.tile([128, MT * BL], F32, tag="gsb")
                nc.scalar.activation(gsb[:], s1[:], act)
                tmp = work.tile([128, MT * BL], F32, tag="tmp")
                nc.vector.tensor_scalar_mul(tmp[:], stT_f32[:], 1.0 - UPDATE)
                nc.vector.scalar_tensor_tensor(
                    stT_f32[:], gsb[:], UPDATE, tmp[:], ALU.mult, ALU.add)
                nc.vector.tensor_copy(stT_bf[:], stT_f32[:])

            def conv_block(i):
                """c[tau*U..][p][u,b] for U steps, bias folded in, f32."""
                xblk = xpool.tile([128, 2 * UB], BF16, tag="xblk")
                for kc in range(2):
                    nc.sync.dma_start(
                        xblk[:, kc * UB:(kc + 1) * UB],
                        xT_d[kc, :, ds(i * UB, UB)])
                cT = cpool.tile([128, U * MT * BL], F32, tag="cT")
                cT_r = cT[:].rearrange("p (u z) -> p u z", z=MT * BL)
                for m in range(MT):
                    pc = psum_c.tile([128, UB], F32, tag="pc")
                    for kc in range(2):
                        nc.tensor.matmul(
                            pc[:],
                            lhsT=wc_sb[:, kc * H + 128 * m:kc * H + 128 * (m + 1)],
                            rhs=xblk[:, kc * UB:(kc + 1) * UB],
                            start=(kc == 0), stop=(kc == 1))
                    # pc free dim is (u, b); scatter to cT cols u*256 + m*16 + b
                    nc.scalar.activation(
                        cT_r[:, :, BL * m:BL * (m + 1)], pc[:],
                        AFT.Identity, bias=bias_sb[:, m:m + 1])
                return cT

            n_iters = T_steps // U
            with tc.For_i(0, n_iters, 1,
                          hint_engines=(mybir.EngineType.PE,)) as i:
                cT = conv_block(i)
                for u in range(U):
                    do_step(cT, u)

            # ---- final output = state @ W_state + bias ----
            sus = mm_phase()
            pT = evict_transpose(sus)
            outf = work.tile([128, MT * BL], F32, tag="outf")
            nc.vector.tensor_tensor(outf[:], pT[:], biasT_sb[:], ALU.add)
            nc.sync.dma_start(outT_d[:], outf[:])
            nc.sync.dma_start(stT_d[:], stT_f32[:])

    nc.compile()
    return nc


def host_inputs(x, W_state, W_conv, bias, T_steps=T_FULL):
    """Per-core input dicts. x: (B, T_steps, D_IN) f32."""
    bf = ml_dtypes.bfloat16
    # W_state [H, H] -> [128, k*H] with w[p, k*H+n] = W[128k+p, n]
    w_arr = np.ascontiguousarray(
        W_state.reshape(KT, 128, H).transpose(1, 0, 2).reshape(128, KT * H)
    ).astype(bf)
    wc_arr = np.ascontiguousarray(
        W_conv.reshape(2, 128, H).transpose(1, 0, 2).reshape(128, 2 * H)
    ).astype(bf)
    bias_arr = np.ascontiguousarray(bias.reshape(MT, 128).T).astype(np.float32)
    bias_bcT = np.repeat(bias.reshape(MT, 128).T[:, :, None], BL,
                         axis=2).reshape(128, MT * BL).astype(np.float32)
    ident = np.eye(128, dtype=np.float32)

    in_maps = []
    for c in range(NCORES):
        xs = x[c * BL:(c + 1) * BL]          # [BL, T, D]
        # xT[kc, p, t*BL+b] = xs[b, t, kc*128+p]
        xT = np.ascontiguousarray(
            xs.reshape(BL, T_steps, 2, 128).transpose(2, 3, 1, 0)
            .reshape(2, 128, T_steps * BL)).astype(bf)
        in_maps.append({
            "xT": xT, "w_arr": w_arr, "wc_arr": wc_arr,
            "bias_arr": bias_arr, "bias_bcT": bias_bcT, "ident": ident,
        })
    return in_maps


def gather_outputs(results):
    """results: list of per-core dicts -> (output, state) full arrays."""
    out = np.empty((B, H), np.float32)
    st = np.empty((B, H), np.float32)
    for c, r in enumerate(results):
        # arr[p, tau*BL+b] = val[b, 128*tau+p]
        o = r["outT"].reshape(128, MT, BL).transpose(2, 1, 0).reshape(BL, H)
        s = r["stT"].reshape(128, MT, BL).transpose(2, 1, 0).reshape(BL, H)
        out[c * BL:(c + 1) * BL] = o
        st[c * BL:(c + 1) * BL] = s
    return out, st


_NC_CACHE = {}


def _get_nc(T_steps=T_FULL, U=8):
    key = (T_steps, U)
    if key not in _NC_CACHE:
        _NC_CACHE[key] = build(T_steps, U)
    return _NC_CACHE[key]


def kernel(x, W_state, W_conv, bias):
    x = np.asarray(x, np.float32)
    W_state = np.asarray(W_state, np.float32)
    W_conv = np.asarray(W_conv, np.float32)
    bias = np.asarray(bias, np.float32)
    nc = _get_nc()
    in_maps = host_inputs(x, W_state, W_conv, bias)
    res = run_bass_kernel_spmd(nc, in_maps, list(range(NCORES)))
    return gather_outputs(res.results)



# revision 2
# speedup vs baseline: 1.2531x; 1.2531x over previous
"""Trainium2 Bass kernel for the CapibaraByte recurrent-scan problem, v2.

Reference computation (B=128, T=1024, D_IN=256, H=2048):
    conv = einsum('btd,dh->bth', x, W_conv)
    step:  s <- 0.9*s + 0.1*gelu(s @ W_state + conv[:,t] + bias)
    out = (s @ W_state + bias, s)

Data-parallel over batch across 8 cores (B_local=16/core); scan fully
on-core.  Per-step GEMM is state-stationary with 4-way PE column tiling
(4 concurrent N=512 streams = the 8-XBUS peak, ~3.4us/step).

v2 changes vs v1:
 - The per-step [b,h] -> [h,b] transpose is done as 4 full 128x128 PE
   transposes of the *packed* PSUM layout (partitions 32g+b) instead of
   16 thin [16,128] transposes: state lives in a padded [128, 512]
   layout col = 128*c + 32*g + b  <->  h = 128*(4g+c) + p, b<16 valid.
 - Per-128-col-chunk software pipeline: evict(ACT) -> transpose(PE) ->
   +conv(DVE) -> gelu(ACT) -> blend(DVE) -> bf16(DVE), with the next
   step's matmuls ordered chunk-major so they start on chunk 0 of the
   new state while chunks 1-3 are still in flight.
 - 0.9*state prescale runs during the matmul phase (off critical path).
 - sW is evicted to bf16 (halves evict cost; product rounding ~0.4%).
"""

import sys

for _p in ("/opt/trn_rl_repo",):
    if _p not in sys.path:
        sys.path.insert(0, _p)

import numpy as np
import ml_dtypes

import concourse.bass as bass
import concourse.tile as tile
from concourse import bacc, mybir
from concourse.bass import ds
from concourse.bass_utils import run_bass_kernel_spmd

AFT = mybir.ActivationFunctionType
ALU = mybir.AluOpType
F32 = mybir.dt.float32
BF16 = mybir.dt.bfloat16

B, T_FULL, D_IN, H = 128, 1024, 256, 2048
NCORES = 8
BL = B // NCORES            # 16 batch rows per core
KT = H // 128               # 16 contraction tiles
MT = H // 128               # 16 output h-tiles
UPDATE = 0.1
NCH = 4                     # 128-col chunks of the padded state layout


def col0(tau):
    """Start column of h-tile tau in the padded [128, 512] state layout."""
    return 128 * (tau % 4) + 32 * (tau // 4)


def build(T_steps=T_FULL, U=8, act=AFT.Gelu_apprx_tanh, static_loop=False):
    assert T_steps % U == 0
    nc = bacc.Bacc("TRN2", target_bir_lowering=False, debug=False,
                   num_devices=NCORES)

    xT_d = nc.dram_tensor("xT", [2, 128, T_steps * BL], BF16,
                          kind="ExternalInput").ap()
    w_d = nc.dram_tensor("w_arr", [128, KT * H], BF16,
                         kind="ExternalInput").ap()
    wc_d = nc.dram_tensor("wc_arr", [128, 2 * H], BF16,
                          kind="ExternalInput").ap()
    biasA_d = nc.dram_tensor("bias_arr", [128, MT], F32,
                             kind="ExternalInput").ap()
    biasT_d = nc.dram_tensor("bias_pad", [128, 512], F32,
                             kind="ExternalInput").ap()
    ident_d = nc.dram_tensor("ident", [128, 128], BF16,
                             kind="ExternalInput").ap()
    outT_d = nc.dram_tensor("outT", [128, 512], F32,
                            kind="ExternalOutput").ap()
    stT_d = nc.dram_tensor("stT", [128, 512], F32,
                           kind="ExternalOutput").ap()

    UB = U * BL  # conv block column count per k-tile
    # matmul k order, chunk-major: state chunk c feeds k-tiles {c, c+4, ...}
    korder = [c + 4 * j for c in range(NCH) for j in range(4)]

    with tile.TileContext(nc) as tc:
        with (
            tc.tile_pool(name="persist", bufs=1) as persist,
            tc.tile_pool(name="xin", bufs=2) as xpool,
            tc.tile_pool(name="cblk", bufs=2) as cpool,
            tc.tile_pool(name="work", bufs=2) as work,
            tc.tile_pool(name="psum_su", bufs=2, space="PSUM") as psum_su,
            tc.tile_pool(name="psum_t", bufs=1, space="PSUM") as psum_t,
            tc.tile_pool(name="psum_c", bufs=2, space="PSUM") as psum_c,
        ):
            # ---- resident tensors ----
            w_sb = persist.tile([128, KT * H], BF16, tag="w_sb")
            nc.sync.dma_start(w_sb[:], w_d[:])
            wc_sb = persist.tile([128, 2 * H], BF16, tag="wc_sb")
            nc.sync.dma_start(wc_sb[:], wc_d[:])
            bias_sb = persist.tile([128, MT], F32, tag="bias_sb")
            nc.sync.dma_start(bias_sb[:], biasA_d[:])
            biasT_sb = persist.tile([128, 512], F32, tag="biasT_sb")
            nc.sync.dma_start(biasT_sb[:], biasT_d[:])
            ident_sb = persist.tile([128, 128], BF16, tag="ident_sb")
            nc.sync.dma_start(ident_sb[:], ident_d[:])

            # state, padded layout [128, 512]: col 128c+32g+b, b<16 valid
            stT_bf = persist.tile([128, 512], BF16, tag="stT_bf")
            nc.vector.memset(stT_bf[:], 0.0)
            stT_f32 = persist.tile([128, 512], F32, tag="stT_f32")
            nc.vector.memset(stT_f32[:], 0.0)

            def mm_phase():
                """64 col-tiled matmuls, chunk-major k order."""
                su = psum_su.tile([128, 512], F32, tag="su")
                for j, k in enumerate(korder):
                    lhs = stT_bf[:, col0(k):col0(k) + BL]
                    for g in range(4):
                        nc.tensor.matmul(
                            su[32 * g:32 * g + BL, :],
                            lhsT=lhs,
                            rhs=w_sb[:, k * H + 512 * g:k * H + 512 * (g + 1)],
                            start=(j == 0), stop=(j == KT - 1),
                            tile_position=(0, 32 * g),
                        )
                return su

            def evict_transpose(su):
                """PSUM packed [32g+b, n] -> bf16 sbuf -> 4 PE transposes."""
                su_bf = work.tile([128, 512], BF16, tag="su_bf")
                pts = []
                for c in range(NCH):
                    nc.scalar.copy(su_bf[:, 128 * c:128 * (c + 1)],
                                   su[:, 128 * c:128 * (c + 1)])
                    pt = psum_t.tile([128, 128], BF16, tag=f"pt{c}",
                                     name=f"pt{c}")
                    nc.tensor.transpose(pt[:],
                                        su_bf[:, 128 * c:128 * (c + 1)],
                                        ident_sb[:])
                    pts.append(pt)
                return pts

            def do_step(cT_r, u):
                # 0.9*state prescale, independent of this step's matmuls
                tmps = []
                for c in range(NCH):
                    tmp = work.tile([128, 128], F32, tag=f"tmp{c}")
                    nc.vector.tensor_scalar_mul(
                        tmp[:], stT_f32[:, 128 * c:128 * (c + 1)], 1.0 - UPDATE)
                    tmps.append(tmp)
                su = mm_phase()
                pts = evict_transpose(su)
                for c in range(NCH):
                    sl = slice(128 * c, 128 * (c + 1))
                    uc = work.tile([128, 128], F32, tag=f"uc{c}")
                    nc.vector.tensor_tensor(uc[:], pts[c][:], cT_r[:, u, sl],
                                            ALU.add)
                    gc = work.tile([128, 128], F32, tag=f"gc{c}")
                    nc.scalar.activation(gc[:], uc[:], act)
                    nc.vector.scalar_tensor_tensor(
                        stT_f32[:, sl], gc[:], UPDATE, tmps[c][:],
                        ALU.mult, ALU.add)
                    nc.vector.tensor_copy(stT_bf[:, sl], stT_f32[:, sl])

            def conv_block(i):
                """conv features for U steps in padded layout, bias folded."""
                xblk = xpool.tile([128, 2 * UB], BF16, tag="xblk")
                for kc in range(2):
                    nc.sync.dma_start(
                        xblk[:, kc * UB:(kc + 1) * UB],
                        xT_d[kc, :, ds(i * UB, UB)])
                cT = cpool.tile([128, U * 512], F32, tag="cT")
                cT_r = cT[:].rearrange("p (u z) -> p u z", z=512)
                for m in range(MT):
                    pc = psum_c.tile([128, UB], F32, tag="pc")
                    for kc in range(2):
                        nc.tensor.matmul(
                            pc[:],
                            lhsT=wc_sb[:, kc * H + 128 * m:kc * H + 128 * (m + 1)],
                            rhs=xblk[:, kc * UB:(kc + 1) * UB],
                            start=(kc == 0), stop=(kc == 1))
                    # pc free dim is (u, b); scatter to cT cols u*512+col0(m)+b
                    nc.scalar.activation(
                        cT_r[:, :, col0(m):col0(m) + BL], pc[:],
                        AFT.Identity, bias=bias_sb[:, m:m + 1])
                return cT_r

            n_iters = T_steps // U
            if static_loop:
                for i in range(n_iters):
                    cT_r = conv_block(i)
                    for u in range(U):
                        do_step(cT_r, u)
            else:
                with tc.For_i(0, n_iters, 1,
                              hint_engines=(mybir.EngineType.PE,)) as i:
                    cT_r = conv_block(i)
                    for u in range(U):
                        do_step(cT_r, u)

            # ---- final output = state @ W_state + bias ----
            su = mm_phase()
            pts = evict_transpose(su)
            outf = work.tile([128, 512], F32, tag="outf")
            for c in range(NCH):
                sl = slice(128 * c, 128 * (c + 1))
                nc.vector.tensor_tensor(outf[:, sl], pts[c][:],
                                        biasT_sb[:, sl], ALU.add)
            nc.sync.dma_start(outT_d[:], outf[:])
            nc.sync.dma_start(stT_d[:], stT_f32[:])

    nc.compile()
    return nc


def host_inputs(x, W_state, W_conv, bias, T_steps=T_FULL):
    """Per-core input dicts. x: (B, T_steps, D_IN) f32."""
    bf = ml_dtypes.bfloat16
    w_arr = np.ascontiguousarray(
        W_state.reshape(KT, 128, H).transpose(1, 0, 2).reshape(128, KT * H)
    ).astype(bf)
    wc_arr = np.ascontiguousarray(
        W_conv.reshape(2, 128, H).transpose(1, 0, 2).reshape(128, 2 * H)
    ).astype(bf)
    bias_arr = np.ascontiguousarray(bias.reshape(MT, 128).T).astype(np.float32)
    # bias_pad[p, 128c+32g+j] = bias[128*(4g+c)+p]
    b16 = bias.reshape(16, 128).T.reshape(128, 4, 4)          # [p, g, c]
    bias_pad = np.ascontiguousarray(
        np.broadcast_to(b16.transpose(0, 2, 1)[:, :, :, None],
                        (128, 4, 4, 32)).reshape(128, 512)
    ).astype(np.float32)
    ident = np.eye(128, dtype=np.float32).astype(bf)

    in_maps = []
    for c in range(NCORES):
        xs = x[c * BL:(c + 1) * BL]          # [BL, T, D]
        xT = np.ascontiguousarray(
            xs.reshape(BL, T_steps, 2, 128).transpose(2, 3, 1, 0)
            .reshape(2, 128, T_steps * BL)).astype(bf)
        in_maps.append({
            "xT": xT, "w_arr": w_arr, "wc_arr": wc_arr,
            "bias_arr": bias_arr, "bias_pad": bias_pad, "ident": ident,
        })
    return in_maps


def _unpad(arr):
    """[128, 512] padded -> [BL, H]: arr[p, 128c+32g+b] = val[b, 128*(4g+c)+p]."""
    return arr.reshape(128, 4, 4, 32).transpose(3, 2, 1, 0).reshape(
        32, H)[:BL]


def gather_outputs(results):
    out = np.empty((B, H), np.float32)
    st = np.empty((B, H), np.float32)
    for c, r in enumerate(results):
        out[c * BL:(c + 1) * BL] = _unpad(r["outT"])
        st[c * BL:(c + 1) * BL] = _unpad(r["stT"])
    return out, st


_NC_CACHE = {}


def _get_nc(T_steps=T_FULL, U=8):
    key = (T_steps, U)
    if key not in _NC_CACHE:
        _NC_CACHE[key] = build(T_steps, U)
    return _NC_CACHE[key]


def kernel(x, W_state, W_conv, bias):
    x = np.asarray(x, np.float32)
    W_state = np.asarray(W_state, np.float32)
    W_conv = np.asarray(W_conv, np.float32)
    bias = np.asarray(bias, np.float32)
    nc = _get_nc()
    in_maps = host_inputs(x, W_state, W_conv, bias)
    res = run_bass_kernel_spmd(nc, in_maps, list(range(NCORES)))
    return gather_outputs(res.results)


# revision 3
# speedup vs baseline: 1.6067x; 1.2822x over previous
"""Trainium2 Bass kernel for the CapibaraByte recurrent-scan problem, v2.

Reference computation (B=128, T=1024, D_IN=256, H=2048):
    conv = einsum('btd,dh->bth', x, W_conv)
    step:  s <- 0.9*s + 0.1*gelu(s @ W_state + conv[:,t] + bias)
    out = (s @ W_state + bias, s)

Data-parallel over batch across 8 cores (B_local=16/core); scan fully
on-core.  Per-step GEMM is state-stationary with 4-way PE column tiling
(4 concurrent N=512 streams = the 8-XBUS peak, ~3.4us/step).

v2 changes vs v1:
 - The per-step [b,h] -> [h,b] transpose is done as 4 full 128x128 PE
   transposes of the *packed* PSUM layout (partitions 32g+b) instead of
   16 thin [16,128] transposes: state lives in a padded [128, 512]
   layout col = 128*c + 32*g + b  <->  h = 128*(4g+c) + p, b<16 valid.
 - Per-128-col-chunk software pipeline: evict(ACT) -> transpose(PE) ->
   +conv(DVE) -> gelu(ACT) -> blend(DVE) -> bf16(DVE), with the next
   step's matmuls ordered chunk-major so they start on chunk 0 of the
   new state while chunks 1-3 are still in flight.
 - 0.9*state prescale runs during the matmul phase (off critical path).
 - sW is evicted to bf16 (halves evict cost; product rounding ~0.4%).
"""

import sys

for _p in ("/opt/trn_rl_repo",):
    if _p not in sys.path:
        sys.path.insert(0, _p)

import numpy as np
import ml_dtypes

import concourse.bass as bass
import concourse.tile as tile
from concourse import bacc, mybir
from concourse.bass import ds
from concourse.bass_utils import run_bass_kernel_spmd

AFT = mybir.ActivationFunctionType
ALU = mybir.AluOpType
F32 = mybir.dt.float32
BF16 = mybir.dt.bfloat16

B, T_FULL, D_IN, H = 128, 1024, 256, 2048
NCORES = 8
BL = B // NCORES            # 16 batch rows per core
KT = H // 128               # 16 contraction tiles
MT = H // 128               # 16 output h-tiles
UPDATE = 0.1
NCH = 4                     # 128-col chunks of the padded state layout


def col0(tau):
    """Start column of h-tile tau in the padded [128, 512] state layout."""
    return 128 * (tau % 4) + 32 * (tau // 4)


def build(T_steps=T_FULL, U=8, act=AFT.Gelu_apprx_tanh, static_loop=False):
    assert T_steps % U == 0
    nc = bacc.Bacc("TRN2", target_bir_lowering=False, debug=False,
                   num_devices=NCORES)

    # padded by 2 conv blocks so the steady-state prefetch of blocks
    # (2j+2, 2j+3) never reads out of range on the final body
    xT_d = nc.dram_tensor("xT", [2, 128, (T_steps + 2 * U) * BL], BF16,
                          kind="ExternalInput").ap()
    w_d = nc.dram_tensor("w_arr", [128, KT * H], BF16,
                         kind="ExternalInput").ap()
    wc_d = nc.dram_tensor("wc_arr", [128, 2 * H], BF16,
                          kind="ExternalInput").ap()
    biasA_d = nc.dram_tensor("bias_arr", [128, MT], F32,
                             kind="ExternalInput").ap()
    biasT_d = nc.dram_tensor("bias_pad", [128, 512], F32,
                             kind="ExternalInput").ap()
    ident_d = nc.dram_tensor("ident", [128, 128], BF16,
                             kind="ExternalInput").ap()
    outT_d = nc.dram_tensor("outT", [128, 512], F32,
                            kind="ExternalOutput").ap()
    stT_d = nc.dram_tensor("stT", [128, 512], F32,
                           kind="ExternalOutput").ap()

    UB = U * BL  # conv block column count per k-tile
    # matmul k order, chunk-major: state chunk c feeds k-tiles {c, c+4, ...}
    korder = [c + 4 * j for c in range(NCH) for j in range(4)]

    with tile.TileContext(nc) as tc:
        with (
            tc.tile_pool(name="persist", bufs=1) as persist,
            tc.tile_pool(name="xin", bufs=2) as xpool,
            tc.tile_pool(name="cblk", bufs=2) as cpool,
            tc.tile_pool(name="work", bufs=2) as work,
            tc.tile_pool(name="psum_su", bufs=2, space="PSUM") as psum_su,
            tc.tile_pool(name="psum_t", bufs=1, space="PSUM") as psum_t,
            tc.tile_pool(name="psum_c", bufs=2, space="PSUM") as psum_c,
        ):
            # ---- resident tensors ----
            w_sb = persist.tile([128, KT * H], BF16, tag="w_sb")
            nc.sync.dma_start(w_sb[:], w_d[:])
            wc_sb = persist.tile([128, 2 * H], BF16, tag="wc_sb")
            nc.sync.dma_start(wc_sb[:], wc_d[:])
            bias_sb = persist.tile([128, MT], F32, tag="bias_sb")
            nc.sync.dma_start(bias_sb[:], biasA_d[:])
            biasT_sb = persist.tile([128, 512], F32, tag="biasT_sb")
            nc.sync.dma_start(biasT_sb[:], biasT_d[:])
            ident_sb = persist.tile([128, 128], BF16, tag="ident_sb")
            nc.sync.dma_start(ident_sb[:], ident_d[:])

            # state, padded layout, split per 128-col chunk so dependency
            # tracking is per-chunk: next step's chunk-c matmuls wait only on
            # chunk c's bf16 copy, not on all four.
            stbf = []
            st32 = []
            for c in range(NCH):
                sb = persist.tile([128, 128], BF16, tag=f"stbf{c}")
                nc.vector.memset(sb[:], 0.0)
                stbf.append(sb)
                s3 = persist.tile([128, 128], F32, tag=f"st32{c}")
                nc.vector.memset(s3[:], 0.0)
                st32.append(s3)

            def mm_phase():
                """64 col-tiled matmuls, chunk-major k order."""
                su = psum_su.tile([128, 512], F32, tag="su")
                for j, k in enumerate(korder):
                    lhs = stbf[k % 4][:, 32 * (k // 4):32 * (k // 4) + BL]
                    for g in range(4):
                        nc.tensor.matmul(
                            su[32 * g:32 * g + BL, :],
                            lhsT=lhs,
                            rhs=w_sb[:, k * H + 512 * g:k * H + 512 * (g + 1)],
                            start=(j == 0), stop=(j == KT - 1),
                            tile_position=(0, 32 * g),
                        )
                return su

            def evict_transpose(su):
                """PSUM packed [32g+b, n] -> bf16 sbuf -> 4 PE transposes."""
                pts = []
                for c in range(NCH):
                    sbf = work.tile([128, 128], BF16, tag=f"subf{c}")
                    nc.scalar.copy(sbf[:], su[:, 128 * c:128 * (c + 1)])
                    pt = psum_t.tile([128, 128], BF16, tag=f"pt{c}",
                                     name=f"pt{c}")
                    nc.tensor.transpose(pt[:], sbf[:], ident_sb[:])
                    pts.append(pt)
                return pts

            def do_step(cT_r, u):
                # 0.9*state prescale, independent of this step's matmuls
                tmps = []
                for c in range(NCH):
                    tmp = work.tile([128, 128], F32, tag=f"tmp{c}")
                    nc.vector.tensor_scalar_mul(
                        tmp[:], st32[c][:], 1.0 - UPDATE)
                    tmps.append(tmp)
                su = mm_phase()
                # per-chunk interleaved chains on per-chunk tiles: chunk 0's
                # chain completes and unblocks next-step chunk-0 matmuls while
                # chunks 1-3 are still in flight
                for c in range(NCH):
                    sl = slice(128 * c, 128 * (c + 1))
                    sbf = work.tile([128, 128], BF16, tag=f"subf{c}")
                    nc.scalar.copy(sbf[:], su[:, sl])
                    pt = psum_t.tile([128, 128], BF16, tag=f"pt{c}",
                                     name=f"pt{c}")
                    nc.tensor.transpose(pt[:], sbf[:], ident_sb[:])
                    uc = work.tile([128, 128], F32, tag=f"uc{c}")
                    nc.vector.tensor_tensor(uc[:], pt[:], cT_r[:, u, sl],
                                            ALU.add)
                    gc = work.tile([128, 128], F32, tag=f"gc{c}")
                    nc.scalar.activation(gc[:], uc[:], act)
                    nc.vector.scalar_tensor_tensor(
                        st32[c][:], gc[:], UPDATE, tmps[c][:],
                        ALU.mult, ALU.add)
                    nc.vector.tensor_copy(stbf[c][:], st32[c][:])

            # ping-pong x staging buffers (persistent so the prefetch DMA of
            # body j+1's blocks can be issued from inside body j)
            xA = persist.tile([128, 2 * UB], BF16, tag="xA")
            xB = persist.tile([128, 2 * UB], BF16, tag="xB")

            def load_x(xt, blk):
                """blk may be a python int or a loop-register expression."""
                for kc in range(2):
                    nc.sync.dma_start(
                        xt[:, kc * UB:(kc + 1) * UB],
                        xT_d[kc, :, ds(blk * UB, UB)])

            def conv_from(xt):
                """conv features for U steps in padded layout, bias folded."""
                cT = cpool.tile([128, U * 512], F32, tag="cT")
                cT_r = cT[:].rearrange("p (u z) -> p u z", z=512)
                for m in range(MT):
                    pc = psum_c.tile([128, UB], F32, tag="pc")
                    for kc in range(2):
                        nc.tensor.matmul(
                            pc[:],
                            lhsT=wc_sb[:, kc * H + 128 * m:kc * H + 128 * (m + 1)],
                            rhs=xt[:, kc * UB:(kc + 1) * UB],
                            start=(kc == 0), stop=(kc == 1))
                    # pc free dim is (u, b); scatter to cT cols u*512+col0(m)+b
                    nc.scalar.activation(
                        cT_r[:, :, col0(m):col0(m) + BL], pc[:],
                        AFT.Identity, bias=bias_sb[:, m:m + 1])
                return cT_r

            n_iters = T_steps // U
            assert n_iters % 2 == 0
            load_x(xA, 0)
            load_x(xB, 1)

            def body(i):
                cT_r = conv_from(xA)
                for u in range(U):
                    do_step(cT_r, u)
                load_x(xA, 2 * i + 2)
                cT_r = conv_from(xB)
                for u in range(U):
                    do_step(cT_r, u)
                load_x(xB, 2 * i + 3)

            if static_loop:
                for i in range(n_iters // 2):
                    body(i)
            else:
                with tc.For_i(0, n_iters // 2, 1,
                              hint_engines=(mybir.EngineType.PE,
                                            mybir.EngineType.DVE)) as i:
                    body(i)

            # ---- final output = state @ W_state + bias ----
            su = mm_phase()
            pts = evict_transpose(su)
            outf = work.tile([128, 512], F32, tag="outf")
            for c in range(NCH):
                sl = slice(128 * c, 128 * (c + 1))
                nc.vector.tensor_tensor(outf[:, sl], pts[c][:],
                                        biasT_sb[:, sl], ALU.add)
            nc.sync.dma_start(outT_d[:], outf[:])
            for c in range(NCH):
                nc.sync.dma_start(stT_d[:, 128 * c:128 * (c + 1)], st32[c][:])

    nc.compile()
    return nc


def host_inputs(x, W_state, W_conv, bias, T_steps=T_FULL):
    """Per-core input dicts. x: (B, T_steps, D_IN) f32."""
    bf = ml_dtypes.bfloat16
    w_arr = np.ascontiguousarray(
        W_state.reshape(KT, 128, H).transpose(1, 0, 2).reshape(128, KT * H)
    ).astype(bf)
    wc_arr = np.ascontiguousarray(
        W_conv.reshape(2, 128, H).transpose(1, 0, 2).reshape(128, 2 * H)
    ).astype(bf)
    bias_arr = np.ascontiguousarray(bias.reshape(MT, 128).T).astype(np.float32)
    # bias_pad[p, 128c+32g+j] = bias[128*(4g+c)+p]
    b16 = bias.reshape(16, 128).T.reshape(128, 4, 4)          # [p, g, c]
    bias_pad = np.ascontiguousarray(
        np.broadcast_to(b16.transpose(0, 2, 1)[:, :, :, None],
                        (128, 4, 4, 32)).reshape(128, 512)
    ).astype(np.float32)
    ident = np.eye(128, dtype=np.float32).astype(bf)

    in_maps = []
    U = 8
    for c in range(NCORES):
        xs = x[c * BL:(c + 1) * BL]          # [BL, T, D]
        xT = np.zeros((2, 128, (T_steps + 2 * U) * BL), dtype=bf)
        xT[:, :, :T_steps * BL] = (
            xs.reshape(BL, T_steps, 2, 128).transpose(2, 3, 1, 0)
            .reshape(2, 128, T_steps * BL).astype(bf))
        in_maps.append({
            "xT": xT, "w_arr": w_arr, "wc_arr": wc_arr,
            "bias_arr": bias_arr, "bias_pad": bias_pad, "ident": ident,
        })
    return in_maps


def _unpad(arr):
    """[128, 512] padded -> [BL, H]: arr[p, 128c+32g+b] = val[b, 128*(4g+c)+p]."""
    return arr.reshape(128, 4, 4, 32).transpose(3, 2, 1, 0).reshape(
        32, H)[:BL]


def gather_outputs(results):
    out = np.empty((B, H), np.float32)
    st = np.empty((B, H), np.float32)
    for c, r in enumerate(results):
        out[c * BL:(c + 1) * BL] = _unpad(r["outT"])
        st[c * BL:(c + 1) * BL] = _unpad(r["stT"])
    return out, st


_NC_CACHE = {}


def _get_nc(T_steps=T_FULL, U=8):
    key = (T_steps, U)
    if key not in _NC_CACHE:
        _NC_CACHE[key] = build(T_steps, U)
    return _NC_CACHE[key]


def kernel(x, W_state, W_conv, bias):
    x = np.asarray(x, np.float32)
    W_state = np.asarray(W_state, np.float32)
    W_conv = np.asarray(W_conv, np.float32)
    bias = np.asarray(bias, np.float32)
    nc = _get_nc()
    in_maps = host_inputs(x, W_state, W_conv, bias)
    res = run_bass_kernel_spmd(nc, in_maps, list(range(NCORES)))
    return gather_outputs(res.results)


# revision 4
# speedup vs baseline: 1.9957x; 1.2422x over previous
"""Trainium2 Bass kernel for the CapibaraByte recurrent-scan problem, v2.

Reference computation (B=128, T=1024, D_IN=256, H=2048):
    conv = einsum('btd,dh->bth', x, W_conv)
    step:  s <- 0.9*s + 0.1*gelu(s @ W_state + conv[:,t] + bias)
    out = (s @ W_state + bias, s)

Data-parallel over batch across 8 cores (B_local=16/core); scan fully
on-core.  Per-step GEMM is state-stationary with 4-way PE column tiling
(4 concurrent N=512 streams = the 8-XBUS peak, ~3.4us/step).

v2 changes vs v1:
 - The per-step [b,h] -> [h,b] transpose is done as 4 full 128x128 PE
   transposes of the *packed* PSUM layout (partitions 32g+b) instead of
   16 thin [16,128] transposes: state lives in a padded [128, 512]
   layout col = 128*c + 32*g + b  <->  h = 128*(4g+c) + p, b<16 valid.
 - Per-128-col-chunk software pipeline: evict(ACT) -> transpose(PE) ->
   +conv(DVE) -> gelu(ACT) -> blend(DVE) -> bf16(DVE), with the next
   step's matmuls ordered chunk-major so they start on chunk 0 of the
   new state while chunks 1-3 are still in flight.
 - 0.9*state prescale runs during the matmul phase (off critical path).
 - sW is evicted to bf16 (halves evict cost; product rounding ~0.4%).
"""

import sys

for _p in ("/opt/trn_rl_repo",):
    if _p not in sys.path:
        sys.path.insert(0, _p)

import numpy as np
import ml_dtypes

import concourse.bass as bass
import concourse.tile as tile
from concourse import bacc, mybir
from concourse.bass import ds
from concourse.bass_utils import run_bass_kernel_spmd

AFT = mybir.ActivationFunctionType
ALU = mybir.AluOpType
F32 = mybir.dt.float32
BF16 = mybir.dt.bfloat16

B, T_FULL, D_IN, H = 128, 1024, 256, 2048
NCORES = 8
BL = B // NCORES            # 16 batch rows per core
KT = H // 128               # 16 contraction tiles
MT = H // 128               # 16 output h-tiles
UPDATE = 0.1
NCH = 4                     # 128-col chunks of the padded state layout


def col0(tau):
    """Start column of h-tile tau in the padded [128, 512] state layout."""
    return 128 * (tau % 4) + 32 * (tau // 4)


def build(T_steps=T_FULL, U=8, act=AFT.Gelu_apprx_tanh, static_loop=False,
          with_bias=False):
    assert T_steps % U == 0
    nc = bacc.Bacc("TRN2", target_bir_lowering=False, debug=False,
                   num_devices=NCORES)

    # padded by 2 conv blocks so the steady-state prefetch of blocks
    # (2j+2, 2j+3) never reads out of range on the final body
    xT_d = nc.dram_tensor("xT", [2, 128, (T_steps + 2 * U) * BL], BF16,
                          kind="ExternalInput").ap()
    w_d = nc.dram_tensor("w_arr", [128, KT * H], BF16,
                         kind="ExternalInput").ap()
    wc_d = nc.dram_tensor("wc_arr", [128, 2 * H], BF16,
                          kind="ExternalInput").ap()
    biasT_d = nc.dram_tensor("bias_pad", [128, 512], F32,
                             kind="ExternalInput").ap()
    ident_d = nc.dram_tensor("ident", [128, 128], BF16,
                             kind="ExternalInput").ap()
    outT_d = nc.dram_tensor("outT", [128, 512], F32,
                            kind="ExternalOutput").ap()
    stT_d = nc.dram_tensor("stT", [128, 512], F32,
                           kind="ExternalOutput").ap()

    UB = U * BL  # conv block column count per k-tile
    # matmul k order, chunk-major: state chunk c feeds k-tiles {c, c+4, ...}
    korder = [c + 4 * j for c in range(NCH) for j in range(4)]

    with tile.TileContext(nc) as tc:
        with (
            tc.tile_pool(name="persist", bufs=1) as persist,
            tc.tile_pool(name="work", bufs=2) as work,
            tc.tile_pool(name="psum_su", bufs=2, space="PSUM") as psum_su,
            tc.tile_pool(name="psum_t", bufs=1, space="PSUM") as psum_t,
        ):
            # ---- resident tensors ----
            w_sb = persist.tile([128, KT * H], BF16, tag="w_sb")
            nc.sync.dma_start(w_sb[:], w_d[:])
            wc_sb = persist.tile([128, 2 * H], BF16, tag="wc_sb")
            nc.sync.dma_start(wc_sb[:], wc_d[:])
            biasT_sb = persist.tile([128, 512], F32, tag="biasT_sb")
            nc.sync.dma_start(biasT_sb[:], biasT_d[:])
            ident_sb = persist.tile([128, 128], BF16, tag="ident_sb")
            nc.sync.dma_start(ident_sb[:], ident_d[:])

            # state, padded layout, split per 128-col chunk so dependency
            # tracking is per-chunk: next step's chunk-c matmuls wait only on
            # chunk c's bf16 copy, not on all four.
            stbf = []
            st32 = []
            for c in range(NCH):
                sb = persist.tile([128, 128], BF16, tag=f"stbf{c}")
                nc.vector.memset(sb[:], 0.0)
                stbf.append(sb)
                s3 = persist.tile([128, 128], F32, tag=f"st32{c}")
                nc.vector.memset(s3[:], 0.0)
                st32.append(s3)

            def mm_phase(xt=None, u=0):
                """Col-tiled matmuls, chunk-major k order.  When xt is given,
                the conv projection x_t @ W_conv is fused in as 2 extra
                k-groups (stationary = x_t^T slice) accumulated into the same
                PSUM bank — first, since x is staged long before the state."""
                su = psum_su.tile([128, 512], F32, tag="su")
                if xt is not None:
                    for kc in range(2):
                        lhs = xt[:, kc * UB + u * BL:kc * UB + (u + 1) * BL]
                        for g in range(4):
                            nc.tensor.matmul(
                                su[32 * g:32 * g + BL, :],
                                lhsT=lhs,
                                rhs=wc_sb[:, kc * H + 512 * g:
                                          kc * H + 512 * (g + 1)],
                                start=(kc == 0), stop=False,
                                tile_position=(0, 32 * g),
                            )
                for j, k in enumerate(korder):
                    lhs = stbf[k % 4][:, 32 * (k // 4):32 * (k // 4) + BL]
                    for g in range(4):
                        nc.tensor.matmul(
                            su[32 * g:32 * g + BL, :],
                            lhsT=lhs,
                            rhs=w_sb[:, k * H + 512 * g:k * H + 512 * (g + 1)],
                            start=(xt is None and j == 0),
                            stop=(j == KT - 1),
                            tile_position=(0, 32 * g),
                        )
                return su

            def evict_transpose(su):
                """PSUM packed [32g+b, n] -> bf16 sbuf -> 4 PE transposes."""
                pts = []
                for c in range(NCH):
                    sbf = work.tile([128, 128], BF16, tag=f"subf{c}")
                    nc.scalar.copy(sbf[:], su[:, 128 * c:128 * (c + 1)])
                    pt = psum_t.tile([128, 128], BF16, tag=f"pt{c}",
                                     name=f"pt{c}")
                    nc.tensor.transpose(pt[:], sbf[:], ident_sb[:])
                    pts.append(pt)
                return pts

            def do_step(xt, u):
                # 0.9*state prescale, independent of this step's matmuls
                tmps = []
                for c in range(NCH):
                    tmp = work.tile([128, 128], F32, tag=f"tmp{c}")
                    nc.vector.tensor_scalar_mul(
                        tmp[:], st32[c][:], 1.0 - UPDATE)
                    tmps.append(tmp)
                su = mm_phase(xt, u)
                # per-chunk interleaved chains on per-chunk tiles: chunk 0's
                # chain completes and unblocks next-step chunk-0 matmuls while
                # chunks 1-3 are still in flight.  conv is already inside su,
                # so without a bias the gelu reads the transposed PSUM
                # directly.  The blend writes the bf16 state (what the next
                # matmul needs) on the critical path; the f32 master is
                # maintained by a duplicate blend off the critical path
                # (same inputs -> numerically identical).
                gcs = []
                for c in range(NCH):
                    sl = slice(128 * c, 128 * (c + 1))
                    sbf = work.tile([128, 128], BF16, tag=f"subf{c}")
                    nc.scalar.copy(sbf[:], su[:, sl])
                    pt = psum_t.tile([128, 128], BF16, tag=f"pt{c}",
                                     name=f"pt{c}")
                    nc.tensor.transpose(pt[:], sbf[:], ident_sb[:])
                    gc = work.tile([128, 128], F32, tag=f"gc{c}")
                    if with_bias:
                        uc = work.tile([128, 128], F32, tag=f"uc{c}")
                        nc.vector.tensor_tensor(uc[:], pt[:], biasT_sb[:, sl],
                                                ALU.add)
                        nc.scalar.activation(gc[:], uc[:], act)
                    else:
                        nc.scalar.activation(gc[:], pt[:], act)
                    nc.vector.scalar_tensor_tensor(
                        stbf[c][:], gc[:], UPDATE, tmps[c][:],
                        ALU.mult, ALU.add)
                    gcs.append(gc)
                for c in range(NCH):
                    nc.vector.scalar_tensor_tensor(
                        st32[c][:], gcs[c][:], UPDATE, tmps[c][:],
                        ALU.mult, ALU.add)

            # ping-pong x staging buffers (persistent so the prefetch DMA of
            # body j+1's blocks can be issued from inside body j)
            xA = persist.tile([128, 2 * UB], BF16, tag="xA")
            xB = persist.tile([128, 2 * UB], BF16, tag="xB")

            def load_x(xt, blk):
                """blk may be a python int or a loop-register expression."""
                for kc in range(2):
                    nc.sync.dma_start(
                        xt[:, kc * UB:(kc + 1) * UB],
                        xT_d[kc, :, ds(blk * UB, UB)])

            n_iters = T_steps // U
            assert n_iters % 2 == 0
            load_x(xA, 0)
            load_x(xB, 1)

            def body(i):
                for u in range(U):
                    do_step(xA, u)
                load_x(xA, 2 * i + 2)
                for u in range(U):
                    do_step(xB, u)
                load_x(xB, 2 * i + 3)

            if static_loop:
                for i in range(n_iters // 2):
                    body(i)
            else:
                with tc.For_i(0, n_iters // 2, 1,
                              hint_engines=(mybir.EngineType.PE,
                                            mybir.EngineType.DVE)) as i:
                    body(i)

            # ---- final output = state @ W_state + bias ----
            su = mm_phase()
            pts = evict_transpose(su)
            outf = work.tile([128, 512], F32, tag="outf")
            for c in range(NCH):
                sl = slice(128 * c, 128 * (c + 1))
                nc.vector.tensor_tensor(outf[:, sl], pts[c][:],
                                        biasT_sb[:, sl], ALU.add)
            nc.sync.dma_start(outT_d[:], outf[:])
            for c in range(NCH):
                nc.sync.dma_start(stT_d[:, 128 * c:128 * (c + 1)], st32[c][:])

    nc.compile()
    return nc


def host_inputs(x, W_state, W_conv, bias, T_steps=T_FULL):
    """Per-core input dicts. x: (B, T_steps, D_IN) f32."""
    bf = ml_dtypes.bfloat16
    w_arr = np.ascontiguousarray(
        W_state.reshape(KT, 128, H).transpose(1, 0, 2).reshape(128, KT * H)
    ).astype(bf)
    wc_arr = np.ascontiguousarray(
        W_conv.reshape(2, 128, H).transpose(1, 0, 2).reshape(128, 2 * H)
    ).astype(bf)
    # bias_pad[p, 128c+32g+j] = bias[128*(4g+c)+p]
    b16 = bias.reshape(16, 128).T.reshape(128, 4, 4)          # [p, g, c]
    bias_pad = np.ascontiguousarray(
        np.broadcast_to(b16.transpose(0, 2, 1)[:, :, :, None],
                        (128, 4, 4, 32)).reshape(128, 512)
    ).astype(np.float32)
    ident = np.eye(128, dtype=np.float32).astype(bf)

    in_maps = []
    U = 8
    for c in range(NCORES):
        xs = x[c * BL:(c + 1) * BL]          # [BL, T, D]
        xT = np.zeros((2, 128, (T_steps + 2 * U) * BL), dtype=bf)
        xT[:, :, :T_steps * BL] = (
            xs.reshape(BL, T_steps, 2, 128).transpose(2, 3, 1, 0)
            .reshape(2, 128, T_steps * BL).astype(bf))
        in_maps.append({
            "xT": xT, "w_arr": w_arr, "wc_arr": wc_arr,
            "bias_pad": bias_pad, "ident": ident,
        })
    return in_maps


def _unpad(arr):
    """[128, 512] padded -> [BL, H]: arr[p, 128c+32g+b] = val[b, 128*(4g+c)+p]."""
    return arr.reshape(128, 4, 4, 32).transpose(3, 2, 1, 0).reshape(
        32, H)[:BL]


def gather_outputs(results):
    out = np.empty((B, H), np.float32)
    st = np.empty((B, H), np.float32)
    for c, r in enumerate(results):
        out[c * BL:(c + 1) * BL] = _unpad(r["outT"])
        st[c * BL:(c + 1) * BL] = _unpad(r["stT"])
    return out, st


_NC_CACHE = {}


def _get_nc(T_steps=T_FULL, U=8, with_bias=False):
    key = (T_steps, U, with_bias)
    if key not in _NC_CACHE:
        _NC_CACHE[key] = build(T_steps, U, with_bias=with_bias)
    return _NC_CACHE[key]


def kernel(x, W_state, W_conv, bias):
    x = np.asarray(x, np.float32)
    W_state = np.asarray(W_state, np.float32)
    W_conv = np.asarray(W_conv, np.float32)
    bias = np.asarray(bias, np.float32)
    # Specialize the build: with a zero bias the gelu reads the transposed
    # PSUM directly (one fewer op + sync hop on the per-step critical path).
    nc = _get_nc(with_bias=bool(np.any(bias)))
    in_maps = host_inputs(x, W_state, W_conv, bias)
    res = run_bass_kernel_spmd(nc, in_maps, list(range(NCORES)))
    return gather_outputs(res.results)
